# revision 34
# baseline (speedup 1.0000x reference)
"""AttentionConv (3x3 local attention, 8 heads) on 8 TRN2 cores.

Data-parallel over batch+rows. The device program (raw Bass blocks with an
explicit mini-scheduler) processes half a sample (28 query rows, 30 input
rows incl. halo) per core per launch; a B=16 call runs four pipelined
launches of 8 half-samples. Per half-sample, tensors are kept channel-major
[C_part, padded_pixel] in SBUF where padded pixel space is a 32x58 grid
(+64-col halo each side) so the 9 window shifts become constant column
offsets. Projections/reductions/broadcasts on TensorE (bf16), elementwise
QK/AV on VectorE, exp/copies on ScalarE. Each launch ends with a device-side
AllGather so every core's DRAM holds the launch's full output.

Host path (the wall clock is dominated by the axon tunnel, not the device):
the PJRT executable is compiled once and cached; x/out travel as bf16 (the
kernel computes in bf16 anyway, so this is numerically free); projection
weights live device-resident between calls; the previous output buffer is
donated as the next call's output scratch (avoids shipping zero buffers);
the replicated AllGathered output is fetched as ONE buffer per launch (the
tunnel charges a large fixed cost per D2H RPC); and the four launches are
pipelined (async dispatch + copy_to_host_async) so chunk N's D2H overlaps
chunk N+1's H2D/exec on the full-duplex tunnel.
"""
import sys

sys.path.insert(0, "/opt/trn_rl_repo")
sys.path.insert(0, "/root/.axon_site/_ro/pypackages")

import numpy as np
import ml_dtypes

import concourse.bass as bass
import concourse.mybir as mybir

BF16 = ml_dtypes.bfloat16
BF = mybir.dt.bfloat16
F32 = mybir.dt.float32
HEADS = 8
H = W = 56
C = 256
HC = C // HEADS
B = 16
SP = 1          # samples per core per launch
NCORES = 8
NCHUNK = B // (NCORES * SP)  # launches per call
NPIX = H * W
PW = 58
PGRID = PW * PW
HALO = 64
PCOLS = PGRID + 2 * HALO
NB = 512
EXP = mybir.ActivationFunctionType.Exp

_CACHE = {}

ENGS = ("sync", "tensor", "scalar", "vector", "gpsimd")


class Sched:
    """Record ops with buffer deps; emit per-engine programs with standalone
    wait_ge instructions (one wait each) and per-instruction sem increments."""

    def __init__(self, nc):
        self.nc = nc
        self.ops = []
        self.deps = []
        self.tick = []
        self.cnt = {e: 0 for e in ENGS}
        self.last_w = {}
        self.readers = {}

    def op(self, eng, emit, reads=(), writes=()):
        i = len(self.ops)
        d = set()
        for b in reads:
            if b in self.last_w:
                d.add(self.last_w[b])
        for b in writes:
            if b in self.last_w:
                d.add(self.last_w[b])
            for r in self.readers.get(b, ()):
                d.add(r)
        self.cnt[eng] += 1
        self.ops.append((eng, emit))
        self.tick.append(self.cnt[eng])
        self.deps.append(d)
        for b in reads:
            self.readers.setdefault(b, []).append(i)
        for b in writes:
            self.last_w[b] = i
            self.readers[b] = []
        return i

    def emit(self, block, sems):
        per_eng = {e: [] for e in ENGS}
        for i, (eng, _) in enumerate(self.ops):
            per_eng[eng].append(i)
        incs = {"sync": 16, "tensor": 1, "scalar": 1, "vector": 1, "gpsimd": 1}

        def run_engine(eng, eproxy):
            observed = {e: 0 for e in ENGS}
            for i in per_eng[eng]:
                need = {}
                for dd in self.deps[i]:
                    de = self.ops[dd][0]
                    if de == eng:
                        continue
                    need[de] = max(need.get(de, 0), self.tick[dd])
                for de, t in need.items():
                    if observed[de] < t:
                        eproxy.wait_ge(sems[de], t * incs[de])
                        observed[de] = t
                ins = self.ops[i][1]()
                ins.then_inc(sems[eng], incs[eng])

        @block.sync
        def _(sync):
            run_engine("sync", sync)
            sync.wait_ge(sems["sync"], self.cnt["sync"] * 16)

        @block.tensor
        def _(tensor):
            run_engine("tensor", tensor)

        @block.scalar
        def _(scalar):
            run_engine("scalar", scalar)

        @block.vector
        def _(vector):
            run_engine("vector", vector)

        @block.gpsimd
        def _(gpsimd):
            run_engine("gpsimd", gpsimd)


def _build_nc(sp=SP, half=False):
    # half=True: each core handles HALF a sample per launch (28 query rows),
    # fed 30 input rows (one halo row each side, zero rows at the sample
    # border supplied by the host). qoff = padded-grid row of query row 0.
    in_rows = 30 if half else H
    qrows = 28 if half else H
    qoff = 2 if half else 1
    gr = in_rows + 2            # padded grid rows (zero row top+bottom)
    pgrid = gr * PW
    pcols = pgrid + 2 * HALO
    npixi = in_rows * W         # x rows per core per rep
    npixq = qrows * W           # out rows per core per rep

    nc = bass.Bass("TRN2", target_bir_lowering=False, num_devices=NCORES)
    x_in = nc.declare_dram_parameter("x", [sp * npixi, C], BF, isOutput=False)
    # Full gathered output on every core: the per-core result is AllGathered
    # device-side so the host fetches ONE replicated buffer (the axon tunnel
    # charges a large fixed cost per D2H RPC, so 8 shard fetches are far
    # slower than 1 full fetch).
    out = nc.declare_dram_parameter("out", [NCORES * sp * npixq, C], BF, isOutput=True)
    out_loc = nc.dram_tensor("out_loc", [sp * npixq, C], BF)
    out_gath = nc.dram_tensor("out_gath", [NCORES * sp * npixq, C], BF, addr_space="Shared")
    fl_in = nc.dram_tensor("fl_in", [8, 128], BF)
    fl_out = nc.dram_tensor("fl_out", [NCORES * 8, 128], BF, addr_space="Shared")
    wts = {}
    for name, mts in (("wq", [128, 128, 72]), ("wk", [128, 128]), ("wv", [128, 128])):
        for ct in range(2):
            for mi, mp in enumerate(mts):
                wname = f"{name}_{ct}_{mi}"
                wts[wname] = nc.declare_dram_parameter(wname, [128, mp], BF, isOutput=False)
    r8_d = [nc.declare_dram_parameter(f"r8_{ct}", [128, 8], BF, isOutput=False) for ct in range(2)]
    b8_d = [nc.declare_dram_parameter(f"b8_{ct}", [8, 128], BF, isOutput=False) for ct in range(2)]
    i72_d = nc.declare_dram_parameter("i72", [72, 72], BF, isOutput=False)
    id128_d = nc.declare_dram_parameter("id128", [128, 128], BF, isOutput=False)

    shifts = [PW * dy + dx - (PW + 1) for dy in range(3) for dx in range(3)]

    import contextlib
    ctx = contextlib.ExitStack()

    _n = [0]

    def sbuf(shape, dt):
        _n[0] += 1
        return ctx.enter_context(nc.sbuf_tensor(f"sb{_n[0]}", shape, dt))

    def psum(shape, dt):
        _n[0] += 1
        return ctx.enter_context(nc.psum_tensor(f"ps{_n[0]}", shape, dt))

    with ctx:
        ctx.enter_context(nc.allow_low_precision(reason="bf16 softmax/AV sums, tol ~1e-2"))
        w_sb = {k: sbuf(list(v.shape), BF) for k, v in wts.items()}
        r8 = [sbuf([128, 8], BF) for _ in range(2)]
        b8 = [sbuf([8, 128], BF) for _ in range(2)]
        i72 = sbuf([72, 72], BF)
        id128 = sbuf([128, 128], BF)
        xT = [sbuf([128, pcols], BF) for _ in range(2)]
        qT = [sbuf([128, pcols], BF) for _ in range(2)]
        kT = [sbuf([128, pcols], BF) for _ in range(2)]
        vT = [sbuf([128, pcols], BF) for _ in range(2)]
        qrT = sbuf([72, pcols], BF)
        U = [sbuf([128, pcols], BF) for _ in range(2)]
        pmb = [sbuf([112, 256], BF) for _ in range(4)]
        flsb = sbuf([8, 128], BF)
        po_pm = sbuf([116, 28 * 256], BF)
        tmp = [[sbuf([128, NB], BF) for _ in range(2)] for _ in range(2)]
        E = [sbuf([8, 9 * NB], BF) for _ in range(2)]
        st = [sbuf([8, NB], BF) for _ in range(4)]
        srt = [sbuf([8, NB], BF) for _ in range(2)]
        abs_ = [sbuf([128, NB], BF) for _ in range(2)]
        mt = [sbuf([128, NB], BF) for _ in range(2)]
        ps_prj = [psum([128, NB], F32) for _ in range(2)]
        ps_lt = [psum([8, NB], F32) for _ in range(2)]
        ps_ab = [psum([128, NB], F32) for _ in range(2)]
        bias30 = sbuf([8, 1], F32)
        ps_t1 = psum([128, 112], BF)
        ps_t4 = psum([116, 128], BF)

        with (
            nc.semaphore("s_sync") as s0, nc.semaphore("s_pe") as s1,
            nc.semaphore("s_act") as s2, nc.semaphore("s_dve") as s3,
            nc.semaphore("s_gp") as s4,
            nc.Block() as block,
        ):
            sems = {"sync": s0, "tensor": s1, "scalar": s2, "vector": s3, "gpsimd": s4}
            S = Sched(nc)

            for k2, d2 in wts.items():
                S.op("sync", lambda t=w_sb[k2], dd=d2: nc.sync.dma_start(out=t[:], in_=dd[:]),
                     writes=(f"w{k2}",))
            for ct in range(2):
                S.op("sync", lambda t=r8[ct], dd=r8_d[ct]: nc.sync.dma_start(out=t[:], in_=dd[:]), writes=(f"r8{ct}",))
                S.op("sync", lambda t=b8[ct], dd=b8_d[ct]: nc.sync.dma_start(out=t[:], in_=dd[:]), writes=(f"b8{ct}",))
            S.op("sync", lambda: nc.sync.dma_start(out=i72[:], in_=i72_d[:]), writes=("i72",))
            S.op("sync", lambda: nc.sync.dma_start(out=id128[:], in_=id128_d[:]), writes=("id128",))
            S.op("vector", lambda: nc.vector.memset(bias30[:], -45.0), writes=("bias30",))
            for ct in range(2):
                S.op("vector", lambda t=xT[ct]: nc.vector.memset(t[:], 0.0), writes=(f"xT{ct}",))
                S.op("vector", lambda t=kT[ct]: nc.vector.memset(t[:], 0.0), writes=(f"kT{ct}",))
                S.op("vector", lambda t=vT[ct]: nc.vector.memset(t[:], 0.0), writes=(f"vT{ct}",))

            for s in range(sp):
                base = s * npixi
                # ---- S1: load x (bf16) tile, transpose to channel-major ----
                for i in range(npixi // 112):
                    pb = pmb[i % 4]
                    pk = f"pmb{i % 4}"
                    S.op("sync", lambda t=pb, r0=base + i * 112:
                         nc.sync.dma_start(out=t[:], in_=x_in[r0:r0 + 112, :]),
                         writes=(pk,))
                    for ct in range(2):
                        S.op("tensor", lambda a=pb, c=ct:
                             nc.tensor.transpose(ps_t1[:], a[:, c * 128:(c + 1) * 128], id128[0:112, 0:112]),
                             reads=(pk, "id128"), writes=("ps_t1",))
                        off = HALO + (2 * i + 1) * PW + 1

                        def cp(c=ct, o=off):
                            dst = xT[c][:, o:o + 2 * PW].rearrange(
                                "p (a b) -> p a b", b=PW)[:, :, 0:W]
                            src = ps_t1[:].rearrange("p (a b) -> p a b", b=W)
                            return nc.scalar.copy(dst, src)
                        S.op("scalar", cp, reads=("ps_t1",), writes=(f"xT{ct}",))

                # ---- S2: projections ----
                projs = [("wq", [(qT[0], "qT0"), (qT[1], "qT1"), (qrT, "qrT")]),
                         ("wk", [(kT[0], "kT0"), (kT[1], "kT1")]),
                         ("wv", [(vT[0], "vT0"), (vT[1], "vT1")])]
                pi = 0
                for nb0 in range(0, pgrid, NB):
                    n = min(NB, pgrid - nb0)
                    col = HALO + nb0
                    for name, dests in projs:
                        for mi, (dest, dkey) in enumerate(dests):
                            mp = dest.shape[0]
                            pp = ps_prj[pi % 2]
                            pk = f"psprj{pi % 2}"
                            pi += 1
                            S.op("tensor", lambda p=pp, m=mp, nn=n, w=w_sb[f"{name}_0_{mi}"], c=col:
                                 nc.tensor.matmul(p[:m, :nn], w[:], xT[0][:, c:c + nn], start=True, stop=False),
                                 reads=(f"w{name}_0_{mi}", "xT0"), writes=(pk,))
                            S.op("tensor", lambda p=pp, m=mp, nn=n, w=w_sb[f"{name}_1_{mi}"], c=col:
                                 nc.tensor.matmul(p[:m, :nn], w[:], xT[1][:, c:c + nn], start=False, stop=True),
                                 reads=(f"w{name}_1_{mi}", "xT1"), writes=(pk,))
                            S.op("scalar", lambda p=pp, m=mp, nn=n, dd=dest, c=col:
                                 nc.scalar.copy(dd[:, c:c + nn], p[:m, :nn]),
                                 reads=(pk,), writes=(dkey,))

                # ---- S3: attention ----
                for bi, nb0 in enumerate(range(0, pgrid, NB)):
                    n = min(NB, pgrid - nb0)
                    r0 = HALO + nb0
                    Eb = E[bi % 2]
                    ek = f"E{bi % 2}"
                    for t in range(9):
                        d = shifts[t]
                        for ct in range(2):
                            S.op("vector", lambda a=tmp[bi % 2][ct], c=ct, nn=n, rr=r0, dd=d:
                                 nc.vector.tensor_mul(a[:, :nn], qT[c][:, rr:rr + nn], kT[c][:, rr + dd:rr + dd + nn]),
                                 reads=(f"qT{ct}", f"kT{ct}"), writes=(f"tmp{bi % 2}{ct}",))
                        lt = ps_lt[t % 2]
                        lk = f"pslt{t % 2}"
                        S.op("tensor", lambda p=lt, nn=n, a=tmp[bi % 2][0]:
                             nc.tensor.matmul(p[:, :nn], r8[0][:], a[:, :nn], start=True, stop=False),
                             reads=(f"tmp{bi % 2}0", "r80"), writes=(lk,))
                        S.op("tensor", lambda p=lt, nn=n, a=tmp[bi % 2][1]:
                             nc.tensor.matmul(p[:, :nn], r8[1][:], a[:, :nn], start=False, stop=False),
                             reads=(f"tmp{bi % 2}1", "r81"), writes=(lk,))
                        S.op("tensor", lambda p=lt, nn=n, tt=t, rr=r0:
                             nc.tensor.matmul(p[:, :nn], i72[:, 8 * tt:8 * tt + 8], qrT[:, rr:rr + nn], start=False, stop=True),
                             reads=("qrT", "i72"), writes=(lk,))
                        S.op("scalar", lambda p=lt, nn=n, tt=t, e=Eb:
                             nc.scalar.activation(e[:, tt * NB:tt * NB + nn], p[:, :nn], EXP, bias=bias30[:]),
                             reads=(lk, "bias30"), writes=(ek,))
                    for j in range(4):
                        S.op("vector", lambda j=j, e=Eb, nn=n:
                             nc.vector.tensor_add(st[j][:, :nn], e[:, 2 * j * NB:2 * j * NB + nn],
                                                  e[:, (2 * j + 1) * NB:(2 * j + 1) * NB + nn]),
                             reads=(ek,), writes=(f"st{j}",))
                    S.op("vector", lambda nn=n: nc.vector.tensor_add(st[0][:, :nn], st[0][:, :nn], st[1][:, :nn]),
                         reads=("st0", "st1"), writes=("st0",))
                    S.op("vector", lambda nn=n: nc.vector.tensor_add(st[2][:, :nn], st[2][:, :nn], st[3][:, :nn]),
                         reads=("st2", "st3"), writes=("st2",))
                    S.op("vector", lambda nn=n: nc.vector.tensor_add(st[0][:, :nn], st[0][:, :nn], st[2][:, :nn]),
                         reads=("st0", "st2"), writes=("st0",))
                    S.op("vector", lambda nn=n, e=Eb: nc.vector.tensor_add(st[0][:, :nn], st[0][:, :nn], e[:, 8 * NB:8 * NB + nn]),
                         reads=("st0", ek), writes=("st0",))
                    sr = srt[bi % 2]
                    S.op("vector", lambda nn=n, r=sr: nc.vector.reciprocal(r[:, :nn], st[0][:, :nn]),
                         reads=("st0",), writes=(f"sr{bi % 2}",))
                    for t in range(9):
                        d = shifts[t]
                        for ct in range(2):
                            ab = ps_ab[ct]
                            ak = f"psab{ct}"
                            S.op("tensor", lambda p=ab, nn=n, c=ct, tt=t, e=Eb:
                                 nc.tensor.matmul(p[:, :nn], b8[c][:], e[:, tt * NB:tt * NB + nn], start=True, stop=True),
                                 reads=(ek, f"b8{ct}"), writes=(ak,))
                            S.op("scalar", lambda p=ab, nn=n, a=abs_[ct]:
                                 nc.scalar.copy(a[:, :nn], p[:, :nn]),
                                 reads=(ak,), writes=(f"abs{ct}",))
                            if t == 0:
                                S.op("vector", lambda nn=n, c=ct, rr=r0, dd=d, a=abs_[ct]:
                                     nc.vector.tensor_mul(U[c][:, rr:rr + nn], a[:, :nn], vT[c][:, rr + dd:rr + dd + nn]),
                                     reads=(f"abs{ct}", f"vT{ct}"), writes=(f"U{ct}",))
                            else:
                                me = "vector" if ct == 0 else "gpsimd"
                                mf = nc.vector.tensor_mul if ct == 0 else nc.gpsimd.tensor_mul
                                S.op(me, lambda nn=n, c=ct, rr=r0, dd=d, a=abs_[ct], m=mt[ct], f=mf:
                                     f(m[:, :nn], a[:, :nn], vT[c][:, rr + dd:rr + dd + nn]),
                                     reads=(f"abs{ct}", f"vT{ct}"), writes=(f"mt{ct}",))
                                S.op("vector", lambda nn=n, c=ct, rr=r0, m=mt[ct]:
                                     nc.vector.tensor_add(U[c][:, rr:rr + nn], U[c][:, rr:rr + nn], m[:, :nn]),
                                     reads=(f"mt{ct}", f"U{ct}"), writes=(f"U{ct}",))
                    for ct in range(2):
                        ab = ps_ab[ct]
                        ak = f"psab{ct}"
                        S.op("tensor", lambda p=ab, nn=n, c=ct, r=sr:
                             nc.tensor.matmul(p[:, :nn], b8[c][:], r[:, :nn], start=True, stop=True),
                             reads=(f"sr{bi % 2}", f"b8{ct}"), writes=(ak,))
                        S.op("vector", lambda p=ab, nn=n, c=ct, rr=r0:
                             nc.vector.tensor_mul(U[c][:, rr:rr + nn], U[c][:, rr:rr + nn], p[:, :nn]),
                             reads=(ak, f"U{ct}"), writes=(f"U{ct}",))

                # ---- S4: transpose back + store (bf16) ----
                obase = s * npixq
                for ch in range(qrows // 2):
                    off = HALO + (2 * ch + qoff) * PW
                    po = po_pm[:, ch * 256:(ch + 1) * 256]
                    for ct in range(2):
                        S.op("tensor", lambda c=ct, o=off:
                             nc.tensor.transpose(ps_t4[:], U[c][:, o:o + 116], id128[:]),
                             reads=(f"U{ct}", "id128"), writes=("ps_t4",))
                        S.op("scalar", lambda p=po, c=ct:
                             nc.scalar.copy(p[:, c * 128:c * 128 + 128], ps_t4[:]),
                             reads=("ps_t4",), writes=(f"po{ch}",))
                    S.op("sync", lambda p=po, r0=obase + 2 * ch * W:
                         nc.sync.dma_start(out=out_loc[r0:r0 + W, :], in_=p[1:57, :]),
                         reads=(f"po{ch}",), writes=("outd",))
                    S.op("sync", lambda p=po, r0=obase + (2 * ch + 1) * W:
                         nc.sync.dma_start(out=out_loc[r0:r0 + W, :], in_=p[59:115, :]),
                         reads=(f"po{ch}",), writes=("outd",))

            S.op("gpsimd", lambda: nc.gpsimd.collective_compute(
                     "AllGather", mybir.AluOpType.bypass,
                     replica_groups=[list(range(NCORES))],
                     ins=[out_loc.ap().opt()], outs=[out_gath.ap().opt()]),
                 reads=("outd",), writes=("ccdone",))
            # Flush collective (defense in depth): NRT runs collectives in
            # order, so this completing adds drain slack for the big gather
            # before the bounce copy below reads it.
            S.op("gpsimd", lambda: nc.gpsimd.collective_compute(
                     "AllGather", mybir.AluOpType.bypass,
                     replica_groups=[list(range(NCORES))],
                     ins=[fl_in.ap().opt()], outs=[fl_out.ap().opt()]),
                 reads=("ccdone",), writes=("ccflush",))
            S.op("sync", lambda: nc.sync.dma_start(out=flsb[:], in_=fl_out[0:8, :]),
                 reads=("ccflush",))
            S.op("sync", lambda: nc.sync.dma_start(out=out[:], in_=out_gath[:]),
                 reads=("ccflush",))

            S.emit(block, sems)
    return nc


def _consts(q_w, k_w, v_w, rel_emb):
    WR = np.zeros((C, 72), np.float32)
    for t in range(9):
        dy, dx = t // 3, t % 3
        for h in range(HEADS):
            WR[h * HC:(h + 1) * HC, t * 8 + h] = rel_emb[dy, dx, h, :]
    wqa = np.concatenate([q_w, q_w @ WR], axis=1)
    ins = {}
    for name, wmat, mts in (("wq", wqa, [128, 128, 72]),
                            ("wk", k_w, [128, 128]), ("wv", v_w, [128, 128])):
        mo = 0
        for mi, mp in enumerate(mts):
            for ct in range(2):
                ins[f"{name}_{ct}_{mi}"] = np.ascontiguousarray(
                    wmat[ct * 128:(ct + 1) * 128, mo:mo + mp]).astype(BF16)
            mo += mp
    for ct in range(2):
        r = np.zeros((128, 8), np.float32)
        for c in range(128):
            r[c, (ct * 128 + c) // HC] = 1.0
        ins[f"r8_{ct}"] = r.astype(BF16)
        ins[f"b8_{ct}"] = np.ascontiguousarray(r.T).astype(BF16)
    ins["i72"] = np.eye(72, dtype=np.float32).astype(BF16)
    ins["id128"] = np.eye(128, dtype=np.float32).astype(BF16)
    return ins


def _setup(sp=SP, half=False):
    """Build the bass program and a cached, compiled PJRT callable for it."""
    import jax
    from jax.sharding import Mesh, PartitionSpec, NamedSharding
    from jax.experimental.shard_map import shard_map
    import concourse.bass2jax as b2j

    b2j.install_neuronx_cc_hook()
    nc = _build_nc(sp, half)

    # Assemble input/output name lists in BIR allocation order (the
    # neuronx_cc_hook parameter-order check requires bass_exec operands to be
    # the jit parameters in order).
    partition_name = nc.partition_id_tensor.name if nc.partition_id_tensor else None
    in_names, out_names, out_avals = [], [], []
    for alloc in nc.m.functions[0].allocations:
        if not isinstance(alloc, mybir.MemoryLocationSet):
            continue
        name = alloc.memorylocations[0].name
        if alloc.kind == "ExternalInput":
            if name != partition_name:
                in_names.append(name)
        elif alloc.kind == "ExternalOutput":
            out_names.append(name)
            out_avals.append(jax.core.ShapedArray(
                tuple(alloc.tensor_shape), mybir.dt.np(alloc.dtype)))
    n_params = len(in_names)
    all_names = in_names + out_names
    if partition_name is not None:
        all_names = all_names + [partition_name]

    def _body(*args):
        operands = list(args)
        if partition_name is not None:
            operands.append(b2j.partition_id_tensor())
        outs = b2j._bass_exec_p.bind(
            *operands,
            out_avals=tuple(out_avals),
            in_names=tuple(all_names),
            out_names=tuple(out_names),
            lowering_input_output_aliases=(),
            sim_require_finite=True,
            sim_require_nnan=True,
            nc=nc,
        )
        return tuple(outs)

    devices = jax.devices()[:NCORES]
    assert len(devices) == NCORES
    mesh = Mesh(np.asarray(devices), ("core",))
    sharding = NamedSharding(mesh, PartitionSpec("core"))
    repl = NamedSharding(mesh, PartitionSpec())
    n_args = n_params + len(out_names)
    # Inputs are sharded over cores; the output (and its donated scratch) is
    # replicated — the NEFF AllGathers the full result onto every core.
    jitted = jax.jit(
        shard_map(_body, mesh=mesh,
                  in_specs=(PartitionSpec("core"),) * n_params
                  + (PartitionSpec(),) * len(out_names),
                  out_specs=(PartitionSpec(),) * len(out_names),
                  check_rep=False),
        donate_argnums=tuple(range(n_params, n_args)),
        keep_unused=True,
    )
    nchunk = (2 * B if half else B) // (NCORES * sp)
    out_shape = tuple(out_avals[0].shape)
    import jax.numpy as jnp
    mk_zeros = jax.jit(
        lambda: jnp.zeros(out_shape, jnp.bfloat16),
        out_shardings=repl)
    return {
        "jax": jax, "jitted": jitted, "sharding": sharding, "repl": repl,
        "mk_zeros": mk_zeros, "sp": sp, "nchunk": nchunk, "half": half,
        "in_names": in_names, "weights": None, "const_dev": None,
        "donate": [None] * nchunk, "xstage": None,
    }


def _bf16_to_f32(a):
    return (a.view(np.uint16).astype(np.uint32) << 16).view(np.float32)


def _run(st, x, q_w, k_w, v_w, rel_emb):
    jax, jitted, sharding = st["jax"], st["jitted"], st["sharding"]
    sp, nchunk = st["sp"], st["nchunk"]

    # Device-resident weights; re-upload only when they change.
    wkey = (np.asarray(q_w, np.float32), np.asarray(k_w, np.float32),
            np.asarray(v_w, np.float32), np.asarray(rel_emb, np.float32))
    prev = st["weights"]
    if prev is None or any(not np.array_equal(a, b) for a, b in zip(prev, wkey)):
        consts = _consts(*wkey)
        tiled = [np.ascontiguousarray(np.tile(consts[n], (NCORES, 1)))
                 for n in st["in_names"] if n != "x"]
        st["const_dev"] = jax.device_put(tiled, sharding)
        st["weights"] = wkey

    # Output scratch buffers to donate (previous outputs, contents ignored:
    # the kernel writes every element). Created device-side — no transfer.
    for c in range(nchunk):
        if st["donate"][c] is None:
            st["donate"][c] = st["mk_zeros"]()

    ys = []
    if st["half"]:
        # Each core gets half a sample: 28 query rows plus one halo row on
        # each side (zero row at the sample border). Chunk l covers samples
        # 4l..4l+3; core k holds sample (8l+k)//2, half k%2. Staging buffers
        # are persistent so the zero border rows stay zero.
        if st["xstage"] is None:
            st["xstage"] = [np.zeros((NCORES, 30, W, C), BF16)
                            for _ in range(nchunk)]
        x4 = x.reshape(B, H, W, C)
        nr = NCORES * 28 * W
        for l in range(nchunk):
            stg = st["xstage"][l]
            for k in range(NCORES):
                hs = NCORES * l + k
                s, hh = hs // 2, hs % 2
                if hh == 0:
                    stg[k, 1:30] = x4[s, 0:29]
                else:
                    stg[k, 0:29] = x4[s, 27:56]
            xd = jax.device_put(stg.reshape(NCORES * 30 * W, C), sharding)
            (y,) = jitted(xd, *st["const_dev"], st["donate"][l])
            # Queue the D2H request now, before the next chunk's upload bytes,
            # so it isn't stuck behind them on the (shared) tunnel stream.
            y.copy_to_host_async()
            ys.append(y)
    else:
        nr = NCORES * sp * NPIX
        xf = x.reshape(B * NPIX, C)
        for c in range(nchunk):
            xc = xf[c * nr:(c + 1) * nr].astype(BF16)
            xd = jax.device_put(xc, sharding)
            (y,) = jitted(xd, *st["const_dev"], st["donate"][c])
            y.copy_to_host_async()
            ys.append(y)
    full = np.empty((B * NPIX, C), np.float32)
    for c in range(nchunk):
        yv = np.asarray(ys[c])
        full[c * nr:(c + 1) * nr] = _bf16_to_f32(yv)
        st["donate"][c] = ys[c]
    return full.reshape(B, H, W, C)


def kernel(x, q_w, k_w, v_w, rel_emb):
    x = np.asarray(x, np.float32)
    assert x.shape == (B, H, W, C)
    if "st" not in _CACHE:
        _CACHE["st"] = _setup(sp=1, half=True)
    return _run(_CACHE["st"], x, q_w, k_w, v_w, rel_emb)


# revision 41
# speedup vs baseline: 1.1185x; 1.1185x over previous
"""AttentionConv (3x3 local attention, 8 heads) on 8 TRN2 cores.

Data-parallel over batch+rows. The device program (raw Bass blocks with an
explicit mini-scheduler) processes half a sample (28 query rows, 30 input
rows incl. halo) per core per launch; a B=16 call runs four pipelined
launches of 8 half-samples. Per half-sample, tensors are kept channel-major
[C_part, padded_pixel] in SBUF where padded pixel space is a 32x58 grid
(+64-col halo each side) so the 9 window shifts become constant column
offsets. Projections/reductions/broadcasts on TensorE (bf16), elementwise
QK/AV on VectorE, exp/copies on ScalarE. Each launch ends with a device-side
AllGather so every core's DRAM holds the launch's full output.

Host path (the wall clock is dominated by the axon tunnel, not the device):
the PJRT executable is compiled once and cached; x/out travel as bf16 (the
kernel computes in bf16 anyway, so this is numerically free); projection
weights live device-resident between calls; the previous output buffer is
donated as the next call's output scratch (avoids shipping zero buffers);
the replicated AllGathered output is fetched as ONE buffer per launch (the
tunnel charges a large fixed cost per D2H RPC); and the four launches are
pipelined (async dispatch + copy_to_host_async) so chunk N's D2H overlaps
chunk N+1's H2D/exec on the full-duplex tunnel.
"""
import sys

sys.path.insert(0, "/opt/trn_rl_repo")
sys.path.insert(0, "/root/.axon_site/_ro/pypackages")

import numpy as np
import ml_dtypes

import concourse.bass as bass
import concourse.mybir as mybir

BF16 = ml_dtypes.bfloat16
BF = mybir.dt.bfloat16
F32 = mybir.dt.float32
HEADS = 8
H = W = 56
C = 256
HC = C // HEADS
B = 16
SP = 1          # samples per core per launch
NCORES = 8
NCHUNK = B // (NCORES * SP)  # launches per call
NPIX = H * W
PW = 58
PGRID = PW * PW
HALO = 64
PCOLS = PGRID + 2 * HALO
NB = 512
EXP = mybir.ActivationFunctionType.Exp

_CACHE = {}

ENGS = ("sync", "tensor", "scalar", "vector", "gpsimd")


class Sched:
    """Record ops with buffer deps; emit per-engine programs with standalone
    wait_ge instructions (one wait each) and per-instruction sem increments."""

    def __init__(self, nc):
        self.nc = nc
        self.ops = []
        self.deps = []
        self.tick = []
        self.cnt = {e: 0 for e in ENGS}
        self.last_w = {}
        self.readers = {}

    def op(self, eng, emit, reads=(), writes=()):
        i = len(self.ops)
        d = set()
        for b in reads:
            if b in self.last_w:
                d.add(self.last_w[b])
        for b in writes:
            if b in self.last_w:
                d.add(self.last_w[b])
            for r in self.readers.get(b, ()):
                d.add(r)
        self.cnt[eng] += 1
        self.ops.append((eng, emit))
        self.tick.append(self.cnt[eng])
        self.deps.append(d)
        for b in reads:
            self.readers.setdefault(b, []).append(i)
        for b in writes:
            self.last_w[b] = i
            self.readers[b] = []
        return i

    def emit(self, block, sems):
        per_eng = {e: [] for e in ENGS}
        for i, (eng, _) in enumerate(self.ops):
            per_eng[eng].append(i)
        incs = {"sync": 16, "tensor": 1, "scalar": 1, "vector": 1, "gpsimd": 1}

        def run_engine(eng, eproxy):
            observed = {e: 0 for e in ENGS}
            for i in per_eng[eng]:
                need = {}
                for dd in self.deps[i]:
                    de = self.ops[dd][0]
                    if de == eng:
                        continue
                    need[de] = max(need.get(de, 0), self.tick[dd])
                for de, t in need.items():
                    if observed[de] < t:
                        eproxy.wait_ge(sems[de], t * incs[de])
                        observed[de] = t
                ins = self.ops[i][1]()
                ins.then_inc(sems[eng], incs[eng])

        @block.sync
        def _(sync):
            run_engine("sync", sync)
            sync.wait_ge(sems["sync"], self.cnt["sync"] * 16)

        @block.tensor
        def _(tensor):
            run_engine("tensor", tensor)

        @block.scalar
        def _(scalar):
            run_engine("scalar", scalar)

        @block.vector
        def _(vector):
            run_engine("vector", vector)

        @block.gpsimd
        def _(gpsimd):
            run_engine("gpsimd", gpsimd)


def _build_nc(sp=SP, half=False):
    # half=True: each core handles HALF a sample per launch (28 query rows),
    # fed 30 input rows (one halo row each side, zero rows at the sample
    # border supplied by the host). qoff = padded-grid row of query row 0.
    in_rows = 30 if half else H
    qrows = 28 if half else H
    qoff = 2 if half else 1
    gr = in_rows + 2            # padded grid rows (zero row top+bottom)
    pgrid = gr * PW
    pcols = pgrid + 2 * HALO
    npixi = in_rows * W         # x rows per core per rep
    npixq = qrows * W           # out rows per core per rep

    nc = bass.Bass("TRN2", target_bir_lowering=False, num_devices=NCORES)
    x_in = nc.declare_dram_parameter("x", [sp * npixi, C], BF, isOutput=False)
    # Full gathered output on every core: the per-core result is AllGathered
    # device-side so the host fetches ONE replicated buffer (the axon tunnel
    # charges a large fixed cost per D2H RPC, so 8 shard fetches are far
    # slower than 1 full fetch).
    out = nc.declare_dram_parameter("out", [NCORES * sp * npixq, C], BF, isOutput=True)
    out_loc = nc.dram_tensor("out_loc", [sp * npixq, C], BF)
    out_gath = nc.dram_tensor("out_gath", [NCORES * sp * npixq, C], BF, addr_space="Shared")
    fl_in = nc.dram_tensor("fl_in", [8, 128], BF)
    fl_out = nc.dram_tensor("fl_out", [NCORES * 8, 128], BF, addr_space="Shared")
    wts = {}
    for name, mts in (("wq", [128, 128, 72]), ("wk", [128, 128]), ("wv", [128, 128])):
        for ct in range(2):
            for mi, mp in enumerate(mts):
                wname = f"{name}_{ct}_{mi}"
                wts[wname] = nc.declare_dram_parameter(wname, [128, mp], BF, isOutput=False)
    r8_d = [nc.declare_dram_parameter(f"r8_{ct}", [128, 8], BF, isOutput=False) for ct in range(2)]
    b8_d = [nc.declare_dram_parameter(f"b8_{ct}", [8, 128], BF, isOutput=False) for ct in range(2)]
    i72_d = nc.declare_dram_parameter("i72", [72, 72], BF, isOutput=False)
    id128_d = nc.declare_dram_parameter("id128", [128, 128], BF, isOutput=False)

    shifts = [PW * dy + dx - (PW + 1) for dy in range(3) for dx in range(3)]

    import contextlib
    ctx = contextlib.ExitStack()

    _n = [0]

    def sbuf(shape, dt):
        _n[0] += 1
        return ctx.enter_context(nc.sbuf_tensor(f"sb{_n[0]}", shape, dt))

    def psum(shape, dt):
        _n[0] += 1
        return ctx.enter_context(nc.psum_tensor(f"ps{_n[0]}", shape, dt))

    with ctx:
        ctx.enter_context(nc.allow_low_precision(reason="bf16 softmax/AV sums, tol ~1e-2"))
        w_sb = {k: sbuf(list(v.shape), BF) for k, v in wts.items()}
        r8 = [sbuf([128, 8], BF) for _ in range(2)]
        b8 = [sbuf([8, 128], BF) for _ in range(2)]
        i72 = sbuf([72, 72], BF)
        id128 = sbuf([128, 128], BF)
        xT = [sbuf([128, pcols], BF) for _ in range(2)]
        qT = [sbuf([128, pcols], BF) for _ in range(2)]
        kT = [sbuf([128, pcols], BF) for _ in range(2)]
        vT = [sbuf([128, pcols], BF) for _ in range(2)]
        qrT = sbuf([72, pcols], BF)
        U = [sbuf([128, pcols], BF) for _ in range(2)]
        pmb = [sbuf([112, 256], BF) for _ in range(4)]
        flsb = sbuf([8, 128], BF)
        po_pm = sbuf([116, 28 * 256], BF)
        tmp = [[sbuf([128, NB], BF) for _ in range(2)] for _ in range(2)]
        E = [sbuf([8, 9 * NB], BF) for _ in range(2)]
        st = [sbuf([8, NB], BF) for _ in range(4)]
        srt = [sbuf([8, NB], BF) for _ in range(2)]
        abs_ = [sbuf([128, NB], BF) for _ in range(2)]
        mt = [sbuf([128, NB], BF) for _ in range(2)]
        ps_prj = [psum([128, NB], F32) for _ in range(2)]
        ps_lt = [psum([8, NB], F32) for _ in range(2)]
        ps_ab = [psum([128, NB], F32) for _ in range(2)]
        bias30 = sbuf([8, 1], F32)
        ps_t1 = psum([128, 112], BF)
        ps_t4 = psum([116, 128], BF)

        with (
            nc.semaphore("s_sync") as s0, nc.semaphore("s_pe") as s1,
            nc.semaphore("s_act") as s2, nc.semaphore("s_dve") as s3,
            nc.semaphore("s_gp") as s4,
            nc.Block() as block,
        ):
            sems = {"sync": s0, "tensor": s1, "scalar": s2, "vector": s3, "gpsimd": s4}
            S = Sched(nc)

            for k2, d2 in wts.items():
                S.op("sync", lambda t=w_sb[k2], dd=d2: nc.sync.dma_start(out=t[:], in_=dd[:]),
                     writes=(f"w{k2}",))
            for ct in range(2):
                S.op("sync", lambda t=r8[ct], dd=r8_d[ct]: nc.sync.dma_start(out=t[:], in_=dd[:]), writes=(f"r8{ct}",))
                S.op("sync", lambda t=b8[ct], dd=b8_d[ct]: nc.sync.dma_start(out=t[:], in_=dd[:]), writes=(f"b8{ct}",))
            S.op("sync", lambda: nc.sync.dma_start(out=i72[:], in_=i72_d[:]), writes=("i72",))
            S.op("sync", lambda: nc.sync.dma_start(out=id128[:], in_=id128_d[:]), writes=("id128",))
            S.op("vector", lambda: nc.vector.memset(bias30[:], -45.0), writes=("bias30",))
            for ct in range(2):
                S.op("vector", lambda t=xT[ct]: nc.vector.memset(t[:], 0.0), writes=(f"xT{ct}",))
                S.op("vector", lambda t=kT[ct]: nc.vector.memset(t[:], 0.0), writes=(f"kT{ct}",))
                S.op("vector", lambda t=vT[ct]: nc.vector.memset(t[:], 0.0), writes=(f"vT{ct}",))

            for s in range(sp):
                base = s * npixi
                # ---- S1: load x (bf16) tile, transpose to channel-major ----
                for i in range(npixi // 112):
                    pb = pmb[i % 4]
                    pk = f"pmb{i % 4}"
                    S.op("sync", lambda t=pb, r0=base + i * 112:
                         nc.sync.dma_start(out=t[:], in_=x_in[r0:r0 + 112, :]),
                         writes=(pk,))
                    for ct in range(2):
                        S.op("tensor", lambda a=pb, c=ct:
                             nc.tensor.transpose(ps_t1[:], a[:, c * 128:(c + 1) * 128], id128[0:112, 0:112]),
                             reads=(pk, "id128"), writes=("ps_t1",))
                        off = HALO + (2 * i + 1) * PW + 1

                        def cp(c=ct, o=off):
                            dst = xT[c][:, o:o + 2 * PW].rearrange(
                                "p (a b) -> p a b", b=PW)[:, :, 0:W]
                            src = ps_t1[:].rearrange("p (a b) -> p a b", b=W)
                            return nc.scalar.copy(dst, src)
                        S.op("scalar", cp, reads=("ps_t1",), writes=(f"xT{ct}",))

                # ---- S2: projections ----
                projs = [("wq", [(qT[0], "qT0"), (qT[1], "qT1"), (qrT, "qrT")]),
                         ("wk", [(kT[0], "kT0"), (kT[1], "kT1")]),
                         ("wv", [(vT[0], "vT0"), (vT[1], "vT1")])]
                pi = 0
                for nb0 in range(0, pgrid, NB):
                    n = min(NB, pgrid - nb0)
                    col = HALO + nb0
                    for name, dests in projs:
                        for mi, (dest, dkey) in enumerate(dests):
                            mp = dest.shape[0]
                            pp = ps_prj[pi % 2]
                            pk = f"psprj{pi % 2}"
                            pi += 1
                            S.op("tensor", lambda p=pp, m=mp, nn=n, w=w_sb[f"{name}_0_{mi}"], c=col:
                                 nc.tensor.matmul(p[:m, :nn], w[:], xT[0][:, c:c + nn], start=True, stop=False),
                                 reads=(f"w{name}_0_{mi}", "xT0"), writes=(pk,))
                            S.op("tensor", lambda p=pp, m=mp, nn=n, w=w_sb[f"{name}_1_{mi}"], c=col:
                                 nc.tensor.matmul(p[:m, :nn], w[:], xT[1][:, c:c + nn], start=False, stop=True),
                                 reads=(f"w{name}_1_{mi}", "xT1"), writes=(pk,))
                            S.op("scalar", lambda p=pp, m=mp, nn=n, dd=dest, c=col:
                                 nc.scalar.copy(dd[:, c:c + nn], p[:m, :nn]),
                                 reads=(pk,), writes=(dkey,))

                # ---- S3: attention ----
                for bi, nb0 in enumerate(range(0, pgrid, NB)):
                    n = min(NB, pgrid - nb0)
                    r0 = HALO + nb0
                    Eb = E[bi % 2]
                    ek = f"E{bi % 2}"
                    for t in range(9):
                        d = shifts[t]
                        for ct in range(2):
                            S.op("vector", lambda a=tmp[bi % 2][ct], c=ct, nn=n, rr=r0, dd=d:
                                 nc.vector.tensor_mul(a[:, :nn], qT[c][:, rr:rr + nn], kT[c][:, rr + dd:rr + dd + nn]),
                                 reads=(f"qT{ct}", f"kT{ct}"), writes=(f"tmp{bi % 2}{ct}",))
                        lt = ps_lt[t % 2]
                        lk = f"pslt{t % 2}"
                        S.op("tensor", lambda p=lt, nn=n, a=tmp[bi % 2][0]:
                             nc.tensor.matmul(p[:, :nn], r8[0][:], a[:, :nn], start=True, stop=False),
                             reads=(f"tmp{bi % 2}0", "r80"), writes=(lk,))
                        S.op("tensor", lambda p=lt, nn=n, a=tmp[bi % 2][1]:
                             nc.tensor.matmul(p[:, :nn], r8[1][:], a[:, :nn], start=False, stop=False),
                             reads=(f"tmp{bi % 2}1", "r81"), writes=(lk,))
                        S.op("tensor", lambda p=lt, nn=n, tt=t, rr=r0:
                             nc.tensor.matmul(p[:, :nn], i72[:, 8 * tt:8 * tt + 8], qrT[:, rr:rr + nn], start=False, stop=True),
                             reads=("qrT", "i72"), writes=(lk,))
                        S.op("scalar", lambda p=lt, nn=n, tt=t, e=Eb:
                             nc.scalar.activation(e[:, tt * NB:tt * NB + nn], p[:, :nn], EXP, bias=bias30[:]),
                             reads=(lk, "bias30"), writes=(ek,))
                    for j in range(4):
                        S.op("vector", lambda j=j, e=Eb, nn=n:
                             nc.vector.tensor_add(st[j][:, :nn], e[:, 2 * j * NB:2 * j * NB + nn],
                                                  e[:, (2 * j + 1) * NB:(2 * j + 1) * NB + nn]),
                             reads=(ek,), writes=(f"st{j}",))
                    S.op("vector", lambda nn=n: nc.vector.tensor_add(st[0][:, :nn], st[0][:, :nn], st[1][:, :nn]),
                         reads=("st0", "st1"), writes=("st0",))
                    S.op("vector", lambda nn=n: nc.vector.tensor_add(st[2][:, :nn], st[2][:, :nn], st[3][:, :nn]),
                         reads=("st2", "st3"), writes=("st2",))
                    S.op("vector", lambda nn=n: nc.vector.tensor_add(st[0][:, :nn], st[0][:, :nn], st[2][:, :nn]),
                         reads=("st0", "st2"), writes=("st0",))
                    S.op("vector", lambda nn=n, e=Eb: nc.vector.tensor_add(st[0][:, :nn], st[0][:, :nn], e[:, 8 * NB:8 * NB + nn]),
                         reads=("st0", ek), writes=("st0",))
                    sr = srt[bi % 2]
                    S.op("vector", lambda nn=n, r=sr: nc.vector.reciprocal(r[:, :nn], st[0][:, :nn]),
                         reads=("st0",), writes=(f"sr{bi % 2}",))
                    for t in range(9):
                        d = shifts[t]
                        for ct in range(2):
                            ab = ps_ab[ct]
                            ak = f"psab{ct}"
                            S.op("tensor", lambda p=ab, nn=n, c=ct, tt=t, e=Eb:
                                 nc.tensor.matmul(p[:, :nn], b8[c][:], e[:, tt * NB:tt * NB + nn], start=True, stop=True),
                                 reads=(ek, f"b8{ct}"), writes=(ak,))
                            S.op("scalar", lambda p=ab, nn=n, a=abs_[ct]:
                                 nc.scalar.copy(a[:, :nn], p[:, :nn]),
                                 reads=(ak,), writes=(f"abs{ct}",))
                            if t == 0:
                                S.op("vector", lambda nn=n, c=ct, rr=r0, dd=d, a=abs_[ct]:
                                     nc.vector.tensor_mul(U[c][:, rr:rr + nn], a[:, :nn], vT[c][:, rr + dd:rr + dd + nn]),
                                     reads=(f"abs{ct}", f"vT{ct}"), writes=(f"U{ct}",))
                            else:
                                me = "vector" if ct == 0 else "gpsimd"
                                mf = nc.vector.tensor_mul if ct == 0 else nc.gpsimd.tensor_mul
                                S.op(me, lambda nn=n, c=ct, rr=r0, dd=d, a=abs_[ct], m=mt[ct], f=mf:
                                     f(m[:, :nn], a[:, :nn], vT[c][:, rr + dd:rr + dd + nn]),
                                     reads=(f"abs{ct}", f"vT{ct}"), writes=(f"mt{ct}",))
                                S.op("vector", lambda nn=n, c=ct, rr=r0, m=mt[ct]:
                                     nc.vector.tensor_add(U[c][:, rr:rr + nn], U[c][:, rr:rr + nn], m[:, :nn]),
                                     reads=(f"mt{ct}", f"U{ct}"), writes=(f"U{ct}",))
                    for ct in range(2):
                        ab = ps_ab[ct]
                        ak = f"psab{ct}"
                        S.op("tensor", lambda p=ab, nn=n, c=ct, r=sr:
                             nc.tensor.matmul(p[:, :nn], b8[c][:], r[:, :nn], start=True, stop=True),
                             reads=(f"sr{bi % 2}", f"b8{ct}"), writes=(ak,))
                        S.op("vector", lambda p=ab, nn=n, c=ct, rr=r0:
                             nc.vector.tensor_mul(U[c][:, rr:rr + nn], U[c][:, rr:rr + nn], p[:, :nn]),
                             reads=(ak, f"U{ct}"), writes=(f"U{ct}",))

                # ---- S4: transpose back + store (bf16) ----
                obase = s * npixq
                for ch in range(qrows // 2):
                    off = HALO + (2 * ch + qoff) * PW
                    po = po_pm[:, ch * 256:(ch + 1) * 256]
                    for ct in range(2):
                        S.op("tensor", lambda c=ct, o=off:
                             nc.tensor.transpose(ps_t4[:], U[c][:, o:o + 116], id128[:]),
                             reads=(f"U{ct}", "id128"), writes=("ps_t4",))
                        S.op("scalar", lambda p=po, c=ct:
                             nc.scalar.copy(p[:, c * 128:c * 128 + 128], ps_t4[:]),
                             reads=("ps_t4",), writes=(f"po{ch}",))
                    S.op("sync", lambda p=po, r0=obase + 2 * ch * W:
                         nc.sync.dma_start(out=out_loc[r0:r0 + W, :], in_=p[1:57, :]),
                         reads=(f"po{ch}",), writes=("outd",))
                    S.op("sync", lambda p=po, r0=obase + (2 * ch + 1) * W:
                         nc.sync.dma_start(out=out_loc[r0:r0 + W, :], in_=p[59:115, :]),
                         reads=(f"po{ch}",), writes=("outd",))

            S.op("gpsimd", lambda: nc.gpsimd.collective_compute(
                     "AllGather", mybir.AluOpType.bypass,
                     replica_groups=[list(range(NCORES))],
                     ins=[out_loc.ap().opt()], outs=[out_gath.ap().opt()]),
                 reads=("outd",), writes=("ccdone",))
            # Flush collective (defense in depth): NRT runs collectives in
            # order, so this completing adds drain slack for the big gather
            # before the bounce copy below reads it.
            S.op("gpsimd", lambda: nc.gpsimd.collective_compute(
                     "AllGather", mybir.AluOpType.bypass,
                     replica_groups=[list(range(NCORES))],
                     ins=[fl_in.ap().opt()], outs=[fl_out.ap().opt()]),
                 reads=("ccdone",), writes=("ccflush",))
            S.op("sync", lambda: nc.sync.dma_start(out=flsb[:], in_=fl_out[0:8, :]),
                 reads=("ccflush",))
            # The bounce copy can rarely catch the gather's remote writes
            # mid-flight (stale bytes from the previous launch). out is
            # write-only, so copy it three times back-to-back on the FIFO DMA
            # queue: each repeat re-reads out_gath a few hundred µs later and
            # overwrites any stale bytes the earlier pass picked up.
            for _rep in range(3):
                S.op("sync", lambda: nc.sync.dma_start(out=out[:], in_=out_gath[:]),
                     reads=("ccflush",), writes=("outw",))

            S.emit(block, sems)
    return nc


def _consts(q_w, k_w, v_w, rel_emb):
    WR = np.zeros((C, 72), np.float32)
    for t in range(9):
        dy, dx = t // 3, t % 3
        for h in range(HEADS):
            WR[h * HC:(h + 1) * HC, t * 8 + h] = rel_emb[dy, dx, h, :]
    wqa = np.concatenate([q_w, q_w @ WR], axis=1)
    ins = {}
    for name, wmat, mts in (("wq", wqa, [128, 128, 72]),
                            ("wk", k_w, [128, 128]), ("wv", v_w, [128, 128])):
        mo = 0
        for mi, mp in enumerate(mts):
            for ct in range(2):
                ins[f"{name}_{ct}_{mi}"] = np.ascontiguousarray(
                    wmat[ct * 128:(ct + 1) * 128, mo:mo + mp]).astype(BF16)
            mo += mp
    for ct in range(2):
        r = np.zeros((128, 8), np.float32)
        for c in range(128):
            r[c, (ct * 128 + c) // HC] = 1.0
        ins[f"r8_{ct}"] = r.astype(BF16)
        ins[f"b8_{ct}"] = np.ascontiguousarray(r.T).astype(BF16)
    ins["i72"] = np.eye(72, dtype=np.float32).astype(BF16)
    ins["id128"] = np.eye(128, dtype=np.float32).astype(BF16)
    return ins


def _setup(sp=SP, half=False):
    """Build the bass program and a cached, compiled PJRT callable for it."""
    import jax
    from jax.sharding import Mesh, PartitionSpec, NamedSharding
    from jax.experimental.shard_map import shard_map
    import concourse.bass2jax as b2j

    b2j.install_neuronx_cc_hook()
    nc = _build_nc(sp, half)

    # Assemble input/output name lists in BIR allocation order (the
    # neuronx_cc_hook parameter-order check requires bass_exec operands to be
    # the jit parameters in order).
    partition_name = nc.partition_id_tensor.name if nc.partition_id_tensor else None
    in_names, out_names, out_avals = [], [], []
    for alloc in nc.m.functions[0].allocations:
        if not isinstance(alloc, mybir.MemoryLocationSet):
            continue
        name = alloc.memorylocations[0].name
        if alloc.kind == "ExternalInput":
            if name != partition_name:
                in_names.append(name)
        elif alloc.kind == "ExternalOutput":
            out_names.append(name)
            out_avals.append(jax.core.ShapedArray(
                tuple(alloc.tensor_shape), mybir.dt.np(alloc.dtype)))
    n_params = len(in_names)
    all_names = in_names + out_names
    if partition_name is not None:
        all_names = all_names + [partition_name]

    def _body(*args):
        operands = list(args)
        if partition_name is not None:
            operands.append(b2j.partition_id_tensor())
        outs = b2j._bass_exec_p.bind(
            *operands,
            out_avals=tuple(out_avals),
            in_names=tuple(all_names),
            out_names=tuple(out_names),
            lowering_input_output_aliases=(),
            sim_require_finite=True,
            sim_require_nnan=True,
            nc=nc,
        )
        return tuple(outs)

    devices = jax.devices()[:NCORES]
    assert len(devices) == NCORES
    mesh = Mesh(np.asarray(devices), ("core",))
    sharding = NamedSharding(mesh, PartitionSpec("core"))
    repl = NamedSharding(mesh, PartitionSpec())
    n_args = n_params + len(out_names)
    # Inputs are sharded over cores; the output (and its donated scratch) is
    # replicated — the NEFF AllGathers the full result onto every core.
    jitted = jax.jit(
        shard_map(_body, mesh=mesh,
                  in_specs=(PartitionSpec("core"),) * n_params
                  + (PartitionSpec(),) * len(out_names),
                  out_specs=(PartitionSpec(),) * len(out_names),
                  check_rep=False),
        donate_argnums=tuple(range(n_params, n_args)),
        keep_unused=True,
    )
    nchunk = (2 * B if half else B) // (NCORES * sp)
    out_shape = tuple(out_avals[0].shape)
    import jax.numpy as jnp
    mk_zeros = jax.jit(
        lambda: jnp.zeros(out_shape, jnp.bfloat16),
        out_shardings=repl)
    return {
        "jax": jax, "jitted": jitted, "sharding": sharding, "repl": repl,
        "mk_zeros": mk_zeros, "sp": sp, "nchunk": nchunk, "half": half,
        "in_names": in_names, "weights": None, "const_dev": None,
        "donate": [None] * nchunk, "xstage": None,
    }


def _bf16_to_f32(a):
    return (a.view(np.uint16).astype(np.uint32) << 16).view(np.float32)


def _run(st, x, q_w, k_w, v_w, rel_emb):
    jax, jitted, sharding = st["jax"], st["jitted"], st["sharding"]
    sp, nchunk = st["sp"], st["nchunk"]

    # Device-resident weights; re-upload only when they change.
    wkey = (np.asarray(q_w, np.float32), np.asarray(k_w, np.float32),
            np.asarray(v_w, np.float32), np.asarray(rel_emb, np.float32))
    prev = st["weights"]
    if prev is None or any(not np.array_equal(a, b) for a, b in zip(prev, wkey)):
        consts = _consts(*wkey)
        tiled = [np.ascontiguousarray(np.tile(consts[n], (NCORES, 1)))
                 for n in st["in_names"] if n != "x"]
        st["const_dev"] = jax.device_put(tiled, sharding)
        st["weights"] = wkey

    # Output scratch buffers to donate (previous outputs, contents ignored:
    # the kernel writes every element). Created device-side — no transfer.
    for c in range(nchunk):
        if st["donate"][c] is None:
            st["donate"][c] = st["mk_zeros"]()

    ys = []
    xds = []
    if st["half"]:
        # Each core gets half a sample: 28 query rows plus one halo row on
        # each side (zero row at the sample border). Chunk l covers samples
        # 4l..4l+3; core k holds sample (8l+k)//2, half k%2. Staging buffers
        # are persistent so the zero border rows stay zero.
        if st["xstage"] is None:
            st["xstage"] = [np.zeros((NCORES, 30, W, C), BF16)
                            for _ in range(nchunk)]
        x4 = x.reshape(B, H, W, C)
        nr = NCORES * 28 * W
        for l in range(nchunk):
            stg = st["xstage"][l]
            for k in range(NCORES):
                hs = NCORES * l + k
                s, hh = hs // 2, hs % 2
                if hh == 0:
                    stg[k, 1:30] = x4[s, 0:29]
                else:
                    stg[k, 0:29] = x4[s, 27:56]
            xd = jax.device_put(stg.reshape(NCORES * 30 * W, C), sharding)
            (y,) = jitted(xd, *st["const_dev"], st["donate"][l])
            # Queue the D2H request now, before the next chunk's upload bytes,
            # so it isn't stuck behind them on the (shared) tunnel stream.
            y.copy_to_host_async()
            xds.append(xd)
            ys.append(y)
    else:
        nr = NCORES * sp * NPIX
        xf = x.reshape(B * NPIX, C)
        for c in range(nchunk):
            xc = xf[c * nr:(c + 1) * nr].astype(BF16)
            xd = jax.device_put(xc, sharding)
            (y,) = jitted(xd, *st["const_dev"], st["donate"][c])
            y.copy_to_host_async()
            xds.append(xd)
            ys.append(y)
    # Fetch + stale-gather guard. The AllGather's bounce copy can rarely ship
    # bytes from the PREVIOUS launch's gather: corrupted output rows are then
    # bit-identical to the previous launch's row at the same position. Detect
    # (whole 512B rows equal ⇒ ~zero false-positive rate) and re-run that
    # launch. xds are not donated, so a retry only needs a fresh out scratch.
    def _has_stale_run(yu, raws, k=64):
        # Stale bytes are an exact copy of another gather's bytes at the same
        # offset: flag any run of >=k consecutive equal elements (chance
        # matches between independent outputs never form such runs).
        for r in raws:
            m = (yu == r).ravel()
            cs = np.cumsum(m, dtype=np.int32)
            if m[:k].all() or (cs[k:] - cs[:-k] == k).any():
                return True
        return False

    full = np.empty((B * NPIX, C), np.float32)
    raws = [] if st.get("prev_raw") is None else [st["prev_raw"]]
    for c in range(nchunk):
        yv = np.asarray(ys[c])
        for _retry in range(3):
            # Stale bytes can only come from the device-previous launch, so
            # the primary pass needs one comparison; a retry's device-previous
            # launch is whatever ran last, so retries check every recent raw.
            chk = raws[-1:] if _retry == 0 else raws
            if not chk or not _has_stale_run(yv.view(np.uint16), chk):
                break
            (y2,) = jitted(xds[c], *st["const_dev"], ys[c])
            y2.copy_to_host_async()
            ys[c] = y2
            yv = np.asarray(y2)
        full[c * nr:(c + 1) * nr] = _bf16_to_f32(yv)
        raws.append(yv.view(np.uint16).copy())
        st["donate"][c] = ys[c]
    st["prev_raw"] = raws[-1]
    return full.reshape(B, H, W, C)


def kernel(x, q_w, k_w, v_w, rel_emb):
    x = np.asarray(x, np.float32)
    assert x.shape == (B, H, W, C)
    if "st" not in _CACHE:
        _CACHE["st"] = _setup(sp=1, half=True)
    return _run(_CACHE["st"], x, q_w, k_w, v_w, rel_emb)
